# revision 20
# baseline (speedup 1.0000x reference)
"""Trainium2 Bass kernel for nn_AttentionModel (masked single-head attention),
seqlength-sparse + load-balanced + fp8 version.

Math (per batch b, X = plms1[b] [S, D]):
    Q = X Wq + bq ; K = X Wk + bk ; Vnb = X Wv          (bias-free V)
    P[s,t] = (Q K^T)[s,t] / sqrt(D),  keys t >= L_b masked
    out = softmax_t(P) @ Vnb + Vnb + 2*bv
(softmax weights sum to 1, so the bv inside attn@(V+bv) + (V+bv) folds into
 the +2*bv term.)

Sparsity: keys t >= L_b contribute nothing, so K-proj / scores / attnV run
only over TT_b = ceil(L_b/128) key tiles instead of S/128.

Sharding (8 cores): batches sorted by TT; the 4 largest ("big") pair with the
4 smallest ("small"), one pair per 2 cores. The big batch is KEY-split: each
core of the pair computes scores/attnV for ALL big queries over half the key
range, producing partial numerators/denominators that the HOST sums. The
small batch is QUERY-split (2 of 4 blocks per core, full key range). The host
performs the final elementwise epilogue out = O/denom + Vnb + 2*bv (all
matmul FLOPs stay on device; the host only reduces/assembles).

Every core runs the SAME program (SPMD); which (batch, rows, keys) a core
handles is determined entirely by how the host packs that core's inputs.
Schedule template (identical on all cores, sized from the actual TT values):
    Q-proj: 6 query blocks of 512 (4 big + 2 small)         [fp8 DoubleRow]
    K-proj: NBS+NSS key slots of 128                        [fp8 DoubleRow]
    V-proj: NBS+NSS+NES value slots of 128                  [bf16]
    scores: (4 blocks x NBS) + (2 blocks x NSS) units       [fp8 DoubleRow]
    attnV : 24 s-tiles x slot-pairs                         [fp8 DoubleRow]
Value slots not needed by attnV are host-remapped to compute the V tiles
beyond the key range that the +V term still needs (zero waste).

fp8 accuracy: exp is computed as exp(score - 3) (denominator uses the same
E so the shift cancels), keeping E within fp8e4m3 range. Simulated end-to-end
rel err 0.0089 vs the 2e-2 gate.
"""

import sys

sys.path.insert(0, "/opt/trn_rl_repo")

import numpy as np
import ml_dtypes

import concourse.bass as bass
import concourse.mybir as mybir
import concourse.tile as tile
from concourse.bass_utils import run_bass_kernel_spmd

try:
    import antenv.axon_hooks  # noqa: F401
except ImportError:
    import types

    _hooks = types.ModuleType("antenv.axon_hooks")
    _hooks._hook = None
    _hooks.set_axon_ntff_profile_hook = lambda h: setattr(_hooks, "_hook", h)
    _hooks.get_axon_ntff_profile_hook = lambda: _hooks._hook
    sys.modules["antenv.axon_hooks"] = _hooks

BF16 = mybir.dt.bfloat16
F32 = mybir.dt.float32
FP8 = mybir.dt.float8e4
NPBF16 = ml_dtypes.bfloat16
NPFP8 = ml_dtypes.float8_e4m3
P = 128
NEG_BIAS = -30000.0
EXP_SHIFT = -3.0
N_CORES = 8
DR = mybir.MatmulPerfMode.DoubleRow


def _split_excess_waits(nc, max_waits=1):
    """walrus rejects instructions with more than a small number of semaphore
    waits; hoist excess waits onto same-engine NOPs placed just before."""
    for f in nc.m.functions:
        for bb in f.blocks:
            out = []
            changed = False
            for ins in bb.instructions:
                si = ins.sync_info
                if si is not None and len(si.on_wait) > max_waits:
                    waits = list(si.on_wait)
                    excess, keep = waits[:-max_waits], waits[-max_waits:]
                    for i in range(0, len(excess), max_waits):
                        nop = mybir.InstNoOp(name=f"{ins.name}-wsplit{i}", ins=[], outs=[])
                        nop.engine = ins.engine
                        nop.sync_info = mybir.SyncInfo(
                            on_wait=excess[i : i + max_waits], on_update=[]
                        )
                        nc.register_instruction(nop)
                        out.append(nop)
                    ins.sync_info = mybir.SyncInfo(
                        on_wait=keep, on_update=list(si.on_update)
                    )
                    changed = True
                out.append(ins)
            if changed:
                bb.instructions = out


def _dedup_ldweights(nc):
    """The PE keeps its loaded stationary across matmuls, but legalization
    emits one InstLdweights per InstMatmult. Drop an Ldweights whose weights
    access pattern is identical to the previous one on the PE stream with
    only matmuls/nops in between: the array already holds those weights.
    Sync info of dropped loads migrates to the next kept instruction."""

    def key(ld):
        a = ld.ins[0]
        return (
            str(getattr(a, "ap", None)), getattr(a, "offset", None),
            str(getattr(a, "dtype", None)), getattr(a, "memref", None),
            str(getattr(ld, "perf_mode", None)),
            str(getattr(ld, "is_transpose", None)),
            str(getattr(ld, "tile_position", None)),
        )

    ndrop = 0
    for f in nc.m.functions:
        for bb in f.blocks:
            out = []
            last_key = None
            pend_waits, pend_updates = [], []
            for ins in bb.instructions:
                if ins.engine != mybir.EngineType.PE:
                    out.append(ins)
                    continue
                if isinstance(ins, mybir.InstLdweights):
                    k = key(ins)
                    if k == last_key:
                        si = ins.sync_info
                        if si is not None:
                            pend_waits.extend(si.on_wait)
                            pend_updates.extend(si.on_update)
                        ndrop += 1
                        continue
                    last_key = k
                elif not isinstance(ins, (mybir.InstMatmult, mybir.InstNoOp)):
                    last_key = None
                if pend_waits or pend_updates:
                    si = ins.sync_info
                    w = list(si.on_wait) if si else []
                    u = list(si.on_update) if si else []
                    ins.sync_info = mybir.SyncInfo(
                        on_wait=pend_waits + w, on_update=pend_updates + u)
                    pend_waits, pend_updates = [], []
                out.append(ins)
            assert not pend_waits and not pend_updates
            bb.instructions = out
    return ndrop


def build_program(S, DIN, DOUT, NBS, NSS, NES):
    """NBS: big key slots (= ceil(maxTTbig/2)); NSS: small key slots
    (= max small TT); NES: extra V-only slots."""
    from contextlib import ExitStack

    KT = DIN // P          # contraction k-tiles (8)
    MT = DOUT // P         # output d m-tiles (8)
    NQB = 6                # query blocks of 512 (4 big + 2 small)
    QCOLS = NQB * 512
    NKS = NBS + NSS        # K / score slots
    NVS = NBS + NSS + NES  # V slots
    NST = NQB * 4          # output s-tiles
    assert KT % 2 == 0 and MT % 2 == 0

    nc = bass.Bass("TRN2", target_bir_lowering=False, debug=False)

    xq8_d = nc.dram_tensor("xq8", [DIN, QCOLS], FP8, kind="ExternalInput").ap()
    xk8_d = nc.dram_tensor("xk8", [DIN, NKS * P], FP8, kind="ExternalInput").ap()
    xkv_d = nc.dram_tensor("xkv", [DIN, NVS * P], BF16, kind="ExternalInput").ap()
    wq8_d = nc.dram_tensor("wq8", [DIN, DOUT], FP8, kind="ExternalInput").ap()
    wk8_d = nc.dram_tensor("wk8", [DIN, DOUT], FP8, kind="ExternalInput").ap()
    wv_d = nc.dram_tensor("wv", [DIN, DOUT], BF16, kind="ExternalInput").ap()
    bqt_d = nc.dram_tensor("bqt", [P, MT], F32, kind="ExternalInput").ap()
    bkt_d = nc.dram_tensor("bkt", [P, MT], F32, kind="ExternalInput").ap()
    mkb_d = nc.dram_tensor("mkb", [P, NKS], F32, kind="ExternalInput").ap()
    outo_d = nc.dram_tensor("outo", [NST * P, DOUT], BF16, kind="ExternalOutput").ap()
    outv_d = nc.dram_tensor("outv", [NVS * P, DOUT], BF16, kind="ExternalOutput").ap()
    outd_d = nc.dram_tensor("outd", [1, NQB * 512], F32, kind="ExternalOutput").ap()

    norm = 1.0 / float(np.sqrt(np.float32(DOUT)))

    with tile.TileContext(nc) as tc, ExitStack() as ctx:
        persist = ctx.enter_context(tc.tile_pool(name="persist", bufs=1))
        q8 = persist.tile([P, MT, QCOLS], FP8)      # Q^T [d, s]
        k8 = persist.tile([P, MT, NKS * P], FP8)    # K^T [d, t]
        vv8 = persist.tile([P, NVS, DOUT], FP8)     # V [t, d] fp8 (attnV)
        bq_sb = persist.tile([P, MT], F32)
        bk_sb = persist.tile([P, MT], F32)
        mk_sb = persist.tile([P, NKS], F32)
        ones8 = persist.tile([P, 2, P], FP8)
        dstage = persist.tile([1, NQB * 512], F32)

        psum = ctx.enter_context(tc.tile_pool(name="psum", bufs=8, space="PSUM"))

        def acc():
            return psum.tile([P, 512], F32, name="acc")

        nc.vector.memset(ones8[:], 1.0)

        # PE warmup to beat the HAM clock ramp + initial DMA window.
        wrm = persist.tile([P, 512], BF16, name="warm")
        nc.vector.memset(wrm[:], 0.0)
        wps = psum.tile([P, 512], F32, name="acc")
        for i in range(24):
            nc.tensor.matmul(wps[:], wrm[:, 0:P], wrm[:], start=(i == 0),
                             stop=(i == 23))

        acc_i = 0

        def epi(dst, ps, bias_ap):
            """PSUM -> dst (+bias), alternating Scalar/Vector."""
            nonlocal acc_i
            if bias_ap is None and acc_i % 2 == 0:
                nc.scalar.copy(dst, ps)
            elif bias_ap is None:
                nc.vector.tensor_copy(dst, ps)
            elif acc_i % 2 == 0:
                nc.scalar.activation(dst, ps,
                                     mybir.ActivationFunctionType.Identity,
                                     bias=bias_ap, scale=1.0)
            else:
                nc.vector.tensor_scalar_add(dst, ps, bias_ap)
            acc_i += 1

        with tc.tile_pool(name="phaseA", bufs=1) as pa:
            xk8_sb = pa.tile([P, KT, NKS * P], FP8)
            xq8_sb = pa.tile([P, KT, QCOLS], FP8)
            xkv_sb = pa.tile([P, KT, NVS * P], BF16)
            wq8_sb = pa.tile([P, KT, DOUT], FP8)
            wk8_sb = pa.tile([P, KT, DOUT], FP8)
            wv_sb = pa.tile([P, KT, DOUT], BF16)
            # K-proj inputs first (smallest: PE starts earliest), then Q, V.
            for k in range(KT):
                nc.sync.dma_start(xk8_sb[:, k, :], xk8_d[k * P:(k + 1) * P, :])
                nc.sync.dma_start(wk8_sb[:, k, :], wk8_d[k * P:(k + 1) * P, :])
            nc.sync.dma_start(bq_sb[:], bqt_d[:])
            nc.sync.dma_start(bk_sb[:], bkt_d[:])
            nc.sync.dma_start(mk_sb[:], mkb_d[:])
            for k in range(KT):
                nc.sync.dma_start(xq8_sb[:, k, :], xq8_d[k * P:(k + 1) * P, :])
                nc.sync.dma_start(wq8_sb[:, k, :], wq8_d[k * P:(k + 1) * P, :])
            for k in range(KT):
                nc.sync.dma_start(xkv_sb[:, k, :], xkv_d[k * P:(k + 1) * P, :])
                nc.sync.dma_start(wv_sb[:, k, :], wv_d[k * P:(k + 1) * P, :])

            # K-proj: per m-tile, accumulate all moving chunks against each
            # Wk k-pair (stationary reused across chunks -> LDW elided).
            kchunks = []
            c0 = 0
            while c0 < NKS * P:
                kchunks.append((c0, min(512, NKS * P - c0)))
                c0 += 512
            for m in range(MT):
                pss = [acc() for _ in kchunks]
                for kp in range(KT // 2):
                    stat = wk8_sb[:, 2 * kp:2 * kp + 2, m * P:(m + 1) * P]
                    for ci, (c0, cw) in enumerate(kchunks):
                        nc.tensor.matmul(
                            pss[ci][:, :cw], stat,
                            xk8_sb[:, 2 * kp:2 * kp + 2, c0:c0 + cw],
                            start=(kp == 0), stop=(kp == KT // 2 - 1),
                            perf_mode=DR)
                for ci, (c0, cw) in enumerate(kchunks):
                    epi(k8[:, m, c0:c0 + cw], pss[ci][:, :cw], bk_sb[:, m:m + 1])

            # Q-proj: per m-tile, two groups of 3 blocks share each Wq k-pair.
            for m in range(MT):
                for g in range(2):
                    qs = range(3 * g, 3 * g + 3)
                    pss = {q: acc() for q in qs}
                    for kp in range(KT // 2):
                        stat = wq8_sb[:, 2 * kp:2 * kp + 2, m * P:(m + 1) * P]
                        for q in qs:
                            nc.tensor.matmul(
                                pss[q][:], stat,
                                xq8_sb[:, 2 * kp:2 * kp + 2, q * 512:(q + 1) * 512],
                                start=(kp == 0), stop=(kp == KT // 2 - 1),
                                perf_mode=DR)
                    for q in qs:
                        epi(q8[:, m, q * 512:(q + 1) * 512], pss[q][:],
                            bq_sb[:, m:m + 1])

            # V-proj: per slot x d-half, bf16; write fp8 (attnV) + bf16 out.
            for v in range(NVS):
                vt = pa.tile([P, DOUT], BF16, name="vstage", bufs=4)
                ph = [acc(), acc()]
                for k in range(KT):
                    stat = xkv_sb[:, k, v * P:(v + 1) * P]
                    for h in range(2):
                        nc.tensor.matmul(
                            ph[h][:], stat,
                            wv_sb[:, k, h * 512:(h + 1) * 512],
                            start=(k == 0), stop=(k == KT - 1))
                for h in range(2):
                    dsl = slice(h * 512, (h + 1) * 512)
                    nc.scalar.copy(vt[:, dsl], ph[h][:])
                    nc.vector.tensor_copy(vv8[:, v, dsl], ph[h][:])
                nc.sync.dma_start(outv_d[v * P:(v + 1) * P, :], vt[:])

        # ---- Phase B ----
        with tc.tile_pool(name="phaseB", bufs=1) as pb:
            e8 = pb.tile([P, NKS, 4 * 512], FP8)
            # scores + exp: big blocks 0..3 x slots 0..NBS-1;
            # small blocks 4,5 x slots NBS..NKS-1 (E cols 0..1023).
            # Small-batch scores first: the attnV phase below runs small
            # s-tiles first (their output DMAs need big-tile matmul shadow
            # to drain), so their E must be ready earliest.
            groups = [(NBS + t, (4, 5)) for t in range(NSS)]
            groups += [(t, (0, 1, 2, 3)) for t in range(NBS)]
            for t, qs in groups:
                pss = {q: acc() for q in qs}
                for mp in range(MT // 2):
                    stat = k8[:, 2 * mp:2 * mp + 2, t * P:(t + 1) * P]
                    for q in qs:
                        nc.tensor.matmul(
                            pss[q][:], stat,
                            q8[:, 2 * mp:2 * mp + 2, q * 512:(q + 1) * 512],
                            start=(mp == 0), stop=(mp == MT // 2 - 1),
                            perf_mode=DR)
                for q in qs:
                    ecol = q * 512 if q < 4 else (q - 4) * 512
                    nc.scalar.activation(
                        e8[:, t, ecol:ecol + 512], pss[q][:],
                        mybir.ActivationFunctionType.Exp,
                        bias=mk_sb[:, t:t + 1], scale=norm)

            # Denominators per block: ones as stationary (one shared LDW),
            # E as moving -> dn[1, 512] accumulated over slot pairs.
            for q in range(NQB):
                if q < 4:
                    t0, tn, ecol = 0, NBS, q * 512
                else:
                    t0, tn, ecol = NBS, NSS, (q - 4) * 512
                dn = acc()
                npair = tn // 2
                for pi in range(npair):
                    ts = t0 + 2 * pi
                    nc.tensor.matmul(dn[:], ones8[:],
                                     e8[:, ts:ts + 2, ecol:ecol + 512],
                                     start=pi == 0,
                                     stop=pi == npair - 1 and tn % 2 == 0,
                                     perf_mode=DR)
                if tn % 2:
                    ts = t0 + tn - 1
                    nc.tensor.matmul(dn[:], ones8[:, 0, :],
                                     e8[:, ts, ecol:ecol + 512],
                                     start=npair == 0, stop=True)
                nc.vector.tensor_copy(dstage[0:1, q * 512:(q + 1) * 512],
                                      dn[0:1, :])

            # attnV: per s-tile, accumulate over slot pairs (E stationary
            # shared by the two d-halves). Small s-tiles first: their
            # epilogues+DMAs come only ~1.3us apart, so put them where the
            # following big tiles' longer matmul runs hide the DMA drain,
            # instead of at the kernel tail.
            for st in list(range(16, NST)) + list(range(16)):
                q, j = st // 4, st % 4
                if q < 4:
                    t0, tn = 0, NBS
                    scol = q * 512 + j * P
                else:
                    t0, tn = NBS, NSS
                    scol = (q - 4) * 512 + j * P
                o0, o1 = acc(), acc()
                npair = tn // 2
                for pi in range(npair):
                    ts = t0 + 2 * pi
                    lhsT = e8[:, ts:ts + 2, scol:scol + P]
                    first, last = pi == 0, pi == npair - 1 and tn % 2 == 0
                    nc.tensor.matmul(o0[:], lhsT, vv8[:, ts:ts + 2, 0:512],
                                     start=first, stop=last, perf_mode=DR)
                    nc.tensor.matmul(o1[:], lhsT, vv8[:, ts:ts + 2, 512:1024],
                                     start=first, stop=last, perf_mode=DR)
                if tn % 2:
                    ts = t0 + tn - 1
                    lhsT1 = e8[:, ts, scol:scol + P]
                    first = npair == 0
                    nc.tensor.matmul(o0[:], lhsT1, vv8[:, ts, 0:512],
                                     start=first, stop=True)
                    nc.tensor.matmul(o1[:], lhsT1, vv8[:, ts, 512:1024],
                                     start=first, stop=True)
                ot = pb.tile([P, DOUT], BF16, name="ostage", bufs=3)
                nc.scalar.copy(ot[:, 0:512], o0[:])
                nc.vector.tensor_copy(ot[:, 512:1024], o1[:])
                nc.sync.dma_start(outo_d[st * P:(st + 1) * P, :], ot[:])
            nc.sync.dma_start(outd_d[:], dstage[0:1, :])

    _dedup_ldweights(nc)
    _split_excess_waits(nc)
    return nc


_PROGRAMS = {}


def _get_program(key):
    if key not in _PROGRAMS:
        _PROGRAMS[key] = build_program(*key)
    return _PROGRAMS[key]


LAST_RESULTS = None


def kernel(plms1, Wq, bq, Wk, bk, Wv, bv, seqlengths):
    global LAST_RESULTS
    plms1, Wq, bq, Wk, bk, Wv, bv, seqlengths = (
        np.asarray(a) for a in (plms1, Wq, bq, Wk, bk, Wv, bv, seqlengths))
    B, S, DIN = plms1.shape
    DOUT = Wq.shape[1]
    assert B == N_CORES
    NT = S // P
    L = [int(x) for x in seqlengths]
    TT = [max(1, min(NT, -(-l // P))) for l in L]

    order = sorted(range(B), key=lambda b: (-TT[b], b))
    bigs, smalls = order[:4], order[4:][::-1]
    NBS = max((TT[b] + 1) // 2 for b in bigs)
    NSS = max(TT[s] for s in smalls)
    used = sum(TT[b] for b in bigs) + 2 * sum(TT[s] for s in smalls)
    extras_n = B * NT - sum(TT)
    NES = max(0, -(-(extras_n - (N_CORES * (NBS + NSS) - used)) // N_CORES))
    NKS, NVS = NBS + NSS, NBS + NSS + NES

    nc = _get_program((S, DIN, DOUT, NBS, NSS, NES))

    # ---- host packing ----
    Wq8 = np.ascontiguousarray(Wq.astype(NPFP8))
    Wk8 = np.ascontiguousarray(Wk.astype(NPFP8))
    Wvb = np.ascontiguousarray(Wv.astype(NPBF16))
    MT = DOUT // P
    bqt = np.ascontiguousarray(bq.astype(np.float32).reshape(MT, P).T)
    bkt = np.ascontiguousarray(bk.astype(np.float32).reshape(MT, P).T)
    XT = [np.ascontiguousarray(plms1[b].T.astype(NPBF16)) for b in range(B)]
    XT8 = [x.astype(NPFP8) for x in XT]

    # V slot maps: per core, slot -> (batch, tile) or None
    vmap = [[None] * NVS for _ in range(N_CORES)]
    vsrc = {}
    core_cfg = []
    for p in range(4):
        big, small = bigs[p], smalls[p]
        ceilb = (TT[big] + 1) // 2
        for role in range(2):
            c = 2 * p + role
            tbase = 0 if role == 0 else ceilb
            nbig = ceilb if role == 0 else TT[big] - ceilb
            core_cfg.append(dict(big=big, small=small, tbase=tbase, nbig=nbig))
            for j in range(nbig):
                vmap[c][j] = (big, tbase + j)
                vsrc[(big, tbase + j)] = (c, j)
            for j in range(TT[small]):
                vmap[c][NBS + j] = (small, j)
                if (small, j) not in vsrc:
                    vsrc[(small, j)] = (c, NBS + j)
    pool = [(b, t) for b in range(B) for t in range(TT[b], NT)]
    pi = 0
    for c in range(N_CORES):
        for sl in range(NVS):
            if vmap[c][sl] is None and pi < len(pool):
                vmap[c][sl] = pool[pi]
                vsrc[pool[pi]] = (c, sl)
                pi += 1
    assert pi == len(pool), f"V slots exhausted: {pi}/{len(pool)}"

    t_idx = np.arange(P)
    in_maps = []
    for c in range(N_CORES):
        cfg = core_cfg[c]
        big, small = cfg["big"], cfg["small"]
        role = c % 2
        # xq8: big blocks 0..3 (all S cols) + small half (1024 cols)
        srow0 = role * (S // 2)
        xq8 = np.concatenate(
            [XT8[big], XT8[small][:, srow0:srow0 + S // 2]], axis=1)
        # xk8 / mkb: NBS big slots + NSS small slots
        xk_cols = np.zeros((DIN, NKS * P), NPFP8)
        mkb = np.full((P, NKS), NEG_BIAS, np.float32)
        for j in range(cfg["nbig"]):
            t = cfg["tbase"] + j
            xk_cols[:, j * P:(j + 1) * P] = XT8[big][:, t * P:(t + 1) * P]
            mkb[:, j] = np.where(t * P + t_idx < L[big], EXP_SHIFT, NEG_BIAS)
        for j in range(TT[small]):
            xk_cols[:, (NBS + j) * P:(NBS + j + 1) * P] = \
                XT8[small][:, j * P:(j + 1) * P]
            mkb[:, NBS + j] = np.where(j * P + t_idx < L[small],
                                       EXP_SHIFT, NEG_BIAS)
        # xkv: V slots
        xkv = np.zeros((DIN, NVS * P), NPBF16)
        for sl in range(NVS):
            if vmap[c][sl] is not None:
                b2, t2 = vmap[c][sl]
                xkv[:, sl * P:(sl + 1) * P] = XT[b2][:, t2 * P:(t2 + 1) * P]
        in_maps.append({
            "xq8": np.ascontiguousarray(xq8),
            "xk8": np.ascontiguousarray(xk_cols),
            "xkv": np.ascontiguousarray(xkv),
            "wq8": Wq8, "wk8": Wk8, "wv": Wvb,
            "bqt": bqt, "bkt": bkt,
            "mkb": np.ascontiguousarray(mkb),
        })

    res = run_bass_kernel_spmd(nc, in_maps, list(range(N_CORES)))
    LAST_RESULTS = res

    # ---- host assembly ----
    bvf = bv.astype(np.float32)
    out = np.empty((B, S, DOUT), np.float32)
    # V tiles
    Vfull = np.empty((B, S, DOUT), np.float32)
    for (b2, t2), (c, sl) in vsrc.items():
        Vfull[b2, t2 * P:(t2 + 1) * P] = \
            res.results[c]["outv"][sl * P:(sl + 1) * P].astype(np.float32)
    for p in range(4):
        big, small = bigs[p], smalls[p]
        A, Bc = res.results[2 * p], res.results[2 * p + 1]
        OB = A["outo"][:S].astype(np.float32) + Bc["outo"][:S].astype(np.float32)
        dB = A["outd"][0, :S] + Bc["outd"][0, :S]
        out[big] = OB / dB[:, None] + Vfull[big] + 2.0 * bvf
        OsA = A["outo"][S:S + S // 2].astype(np.float32)
        OsB = Bc["outo"][S:S + S // 2].astype(np.float32)
        dsA = A["outd"][0, S:S + S // 2]
        dsB = Bc["outd"][0, S:S + S // 2]
        Os = np.concatenate([OsA / dsA[:, None], OsB / dsB[:, None]], axis=0)
        out[small] = Os + Vfull[small] + 2.0 * bvf
    return out
